# revision 50
# baseline (speedup 1.0000x reference)
"""Trainium2 Bass kernel for nn_MinibatchDiscrimination (v2, symmetric).

Reference math:
    m = (x @ T).reshape(B, 64, 16)                      # B=512
    D[i, j, o] = sum_k |m[i,o,k] - m[j,o,k]|
    out[i, o] = sum_j exp(-D[i,j,o])
    return concat([x, out], axis=1)                     # [512, 2112]

Device strategy (8 NeuronCores):
  exp(-D) is symmetric in (i, j), so each unordered pair is evaluated
  once and contributes to BOTH out[i] and out[j].  Core c receives x^T
  with rows rotated so its own 64 rows sit at local columns 0..63, and
  computes D only against the W=320-column window (own block + the next
  4 blocks of 64, wrapping).  Block-distance 1..3 pairs are covered by
  exactly one core; distance-4 pairs are covered by two cores (c and
  c+4), which is made exact by adding ln(2) to D for window columns
  256..320 so each core contributes exactly half of exp(-D).  Per own
  row the device produces the window row-sum (accum_out of the Exp
  activation); the cross contributions sum_{i in core} exp(-D[i, j])
  for window columns 64..320 are accumulated on DVE in bf16 (all those
  values underflow-scale ~e^-800), folded over the two partition halves
  by one PE matmul, and added to the other rows' outputs on the host.

  On-device pipeline per core: m^T in layout [(o,k) chunks of 128, 320]
  via fp8 matmuls; per own row i the L1 distance uses the relu
  decomposition  sum_k |d_k| = S_i - S_j + 2*sum_k relu(m_j - m_i)
  (no fused |a-b| op on this walrus), with relu tiles produced by
  ScalarE (Relu activation, fp8 out) and VectorE (2-scalar tensor_scalar
  (add, max) — 2x DVE perf mode; scalar_tensor_tensor is 1x-only) on a
  per-(chunk, half) slot split; TensorE reduces over k with one-hot
  2.0-selection matmuls into PSUM, two rows (i, i+32) interleaved in the
  two PSUM partition halves so their matmuls overlap in distinct PE
  column groups.  One Exp activation per row pair (scale=-1, bias
  -bf16(S_i)) emits the row sums via accum_out.
  The diagonal D_ii is exactly 0: relu terms vanish identically (both
  operands read the same bf16 values) and the -bf16(S_j) correction
  cancels the -bf16(S_i) exp bias exactly.
  Raw bass (explicit engine blocks + standalone semaphore waits): the
  walrus rejects instructions with >1 inline sync-wait.
"""

import math
import os
import sys
from contextlib import ExitStack

import numpy as np

sys.path.insert(0, "/opt/trn_rl_repo")

import concourse.bass as bass  # noqa: E402
import concourse.mybir as mybir  # noqa: E402
from concourse.bass_utils import run_bass_kernel_spmd  # noqa: E402

import ml_dtypes  # noqa: E402

P = 128
B = 512
DIM = 2048
OF = 64  # out features
KD = 16  # kernel dim
OK = OF * KD  # 1024
NCORES = 8
ROWS = B // NCORES  # 64 own rows per core
W = 320  # m columns (union of the two halves' windows)
WT = 288  # per-half window width (own 32-block + 8 more 32-blocks)
HS = 32  # window shift of the second half (rows 32..63)
TRW = 256  # transpose-partial columns (window minus own 32-block)
NCH = OK // P  # 8 (o,k)-chunks
NDC2 = DIM // (2 * P)  # 8 DoubleRow contraction chunks (256 rows each)
NPAIRS = ROWS // 2  # 32 ip iterations (2 rows per ip)
NSLOT = 2 * NCH  # 16 (chunk, half) slots per ip

ACT_SLOTS = int(os.environ.get("KERNEL_ACT_SLOTS", "5"))  # slots on ScalarE
NB8 = int(os.environ.get("KERNEL_NB8", "15"))  # fp8 abs tile ring size
NBB = int(os.environ.get("KERNEL_NBB", "33"))  # bf16 abs tile ring size
ED = int(os.environ.get("KERNEL_ED", "2"))  # exp emitted ED ips late
EDA = ED + 1  # E-accumulate emitted EDA ips late
NDP = 4

BF16 = mybir.dt.bfloat16
F32 = mybir.dt.float32
FP8 = mybir.dt.float8e5  # e5m2

LN2 = math.log(2.0)

last_exec_time_ns = None

_cached = {}


def _install_ntff_hook():
    """The agent image's `antenv` lacks `axon_hooks`; recreate the NTFF
    profile hook via ctypes against libaxon_pjrt.so and keep artifacts
    local."""
    import contextlib
    import ctypes
    import types

    try:
        import antenv.axon_hooks  # noqa: F401

        return True
    except ImportError:
        pass

    so_path = "/opt/axon/libaxon_pjrt.so"
    if not os.path.exists(so_path):
        return False
    lib = ctypes.CDLL(so_path)
    if not hasattr(lib, "axon_start_nrt_profile"):
        return False
    lib.axon_start_nrt_profile.argtypes = [
        ctypes.POINTER(ctypes.c_int64),
        ctypes.c_size_t,
    ]
    lib.axon_start_nrt_profile.restype = ctypes.c_int64
    lib.axon_stop_nrt_profile.argtypes = [ctypes.c_char_p]
    lib.axon_stop_nrt_profile.restype = ctypes.c_int64

    @contextlib.contextmanager
    def _hook(output_dir, device_ids):
        import jax

        jax.devices()
        if device_ids:
            ids = (ctypes.c_int64 * len(device_ids))(*device_ids)
            rc = lib.axon_start_nrt_profile(ids, len(device_ids))
        else:
            rc = lib.axon_start_nrt_profile(None, 0)
        if rc != 0:
            raise RuntimeError(f"axon_start_nrt_profile rc={rc}")
        try:
            yield
        finally:
            n = lib.axon_stop_nrt_profile(str(output_dir).encode())
            print(f"ntff profile: {n} file(s) written to {output_dir}", file=sys.stderr)

    mod = types.ModuleType("antenv.axon_hooks")
    _state = {"hook": _hook}
    mod.set_axon_ntff_profile_hook = lambda h: _state.__setitem__("hook", h)
    mod.get_axon_ntff_profile_hook = lambda: _state["hook"]
    import antenv

    sys.modules["antenv.axon_hooks"] = mod
    antenv.axon_hooks = mod

    import concourse.bass_utils as bu

    bu.upload_artifacts = lambda tmpdir: str(tmpdir)
    return True


class _WaitTracker:
    """Emit a standalone wait only when this engine hasn't already
    waited for (at least) the needed value on that semaphore."""

    def __init__(self, eng):
        self.eng = eng
        self.seen = {}

    def wait_ge(self, sem, val):
        if self.seen.get(sem.num, -1) >= val:
            return
        self.eng.wait_ge(sem, val)
        self.seen[sem.num] = val


MM_PER_IP = 1 + NSLOT  # 1 correction (both halves) + 16 slot matmuls


def _slot_layout(act_slots=ACT_SLOTS):
    """slot s = (chunk, half); choose which slots run on ScalarE
    (spread across the slot sequence), the rest on VectorE."""
    slots = [(c, h) for c in range(NCH) for h in range(2)]
    act_idx = sorted({round(i * (NSLOT - 1) / max(act_slots - 1, 1)) for i in range(act_slots)}) if act_slots else []
    # ensure exactly act_slots distinct indices
    i = 0
    while len(act_idx) < act_slots:
        if i not in act_idx:
            act_idx.append(i)
        i += 1
    act_idx = sorted(act_idx[:act_slots])
    dve_idx = [s for s in range(NSLOT) if s not in act_idx]
    return slots, act_idx, dve_idx


def _build_nc(act_slots=ACT_SLOTS):
    nc = bass.Bass()
    AF = mybir.ActivationFunctionType
    ALU = mybir.AluOpType

    slots, act_idx, dve_idx = _slot_layout(act_slots)
    SA = len(act_idx)
    SD = len(dve_idx)
    assert SD >= 8, "E-add same-engine spacing relies on >=8 DVE ops per ip"
    a_pos = {s: n for n, s in enumerate(act_idx)}  # slot -> per-ip act index
    d_pos = {s: n for n, s in enumerate(dve_idx)}

    # phase-1 inputs in DoubleRow interleave: row (dcp*128+p) holds the
    # two contraction rows (dcp*256+2p, dcp*256+2p+1) concatenated
    xT = nc.declare_dram_parameter("xT", [NDC2 * P, 2 * W], FP8, isOutput=False)
    Tw = nc.declare_dram_parameter("Tw", [NDC2 * P, 2 * OK], FP8, isOutput=False)
    sel8 = nc.declare_dram_parameter("sel8", [P, NCH * OF], FP8, isOutput=False)
    selb = nc.declare_dram_parameter("selb", [P, NCH * OF], BF16, isOutput=False)
    sel1b = nc.declare_dram_parameter("sel1b", [P, NCH * OF], BF16, isOutput=False)
    identw = nc.declare_dram_parameter("identw", [P, P], BF16, isOutput=False)
    out_d = nc.declare_dram_parameter("out", [P, NPAIRS], F32, isOutput=True)
    tro_d = nc.declare_dram_parameter("tro", [P, TRW], F32, isOutput=True)

    ctx = ExitStack()
    with ctx:
        tw_t = [ctx.enter_context(nc.sbuf_tensor(f"tw{i}", [P, 2, OK], FP8)) for i in range(NDC2)]
        xt_t = [ctx.enter_context(nc.sbuf_tensor(f"xt{i}", [P, 2, W], FP8)) for i in range(NDC2)]
        m_t = [ctx.enter_context(nc.sbuf_tensor(f"m{i}", [P, W], BF16)) for i in range(NCH)]
        mon_t = [ctx.enter_context(nc.sbuf_tensor(f"mon{i}", [P, ROWS], F32)) for i in range(NCH)]
        sel8_t = ctx.enter_context(nc.sbuf_tensor("sel8t", [P, NCH * OF], FP8))
        selb_t = ctx.enter_context(nc.sbuf_tensor("selbt", [P, NCH * OF], BF16))
        sel1b_t = ctx.enter_context(nc.sbuf_tensor("sel1bt", [P, NCH * OF], BF16))
        identw_t = ctx.enter_context(nc.sbuf_tensor("identwt", [P, P], BF16))
        abs8_t = [ctx.enter_context(nc.sbuf_tensor(f"abs8_{i}", [P, WT], FP8)) for i in range(NB8)]
        absb_t = [ctx.enter_context(nc.sbuf_tensor(f"absb_{i}", [P, WT], BF16)) for i in range(NBB)]
        nsful_t = ctx.enter_context(nc.sbuf_tensor("nsful", [P, WT], BF16))
        sbias_t = ctx.enter_context(nc.sbuf_tensor("sbias", [P, NPAIRS], F32))
        stmp_t = ctx.enter_context(nc.sbuf_tensor("stmp", [OF, ROWS], BF16))
        esc_t = [ctx.enter_context(nc.sbuf_tensor(f"esct{i}", [P, WT], BF16)) for i in range(2)]
        eacc_t = [ctx.enter_context(nc.sbuf_tensor(f"eacc{i}", [P, TRW], BF16)) for i in range(2)]
        osb_t = ctx.enter_context(nc.sbuf_tensor("osbt", [P, NPAIRS], F32))
        tro_t = ctx.enter_context(nc.sbuf_tensor("trot", [P, TRW], F32))

        ps_t = [ctx.enter_context(nc.psum_tensor(f"ps{i}", [P, W], F32)) for i in range(2)]
        dp_t = [ctx.enter_context(nc.psum_tensor(f"dp{i}", [P, WT], F32)) for i in range(NDP)]
        pss_t = ctx.enter_context(nc.psum_tensor("pss", [OF, W], F32))

        dmag = [ctx.enter_context(nc.semaphore(f"dmag{i}")) for i in range(5)]
        dma_cnt = ctx.enter_context(nc.semaphore("dma_cnt"))
        mm_done = ctx.enter_context(nc.semaphore("mm_done"))
        m_copied = ctx.enter_context(nc.semaphore("m_copied"))
        s_done = ctx.enter_context(nc.semaphore("s_done"))
        s_copied = ctx.enter_context(nc.semaphore("s_copied"))
        pe_abs = ctx.enter_context(nc.semaphore("pe_abs"))
        act_abs = ctx.enter_context(nc.semaphore("act_abs"))
        dve_abs = ctx.enter_context(nc.semaphore("dve_abs"))
        exp_done = ctx.enter_context(nc.semaphore("exp_done"))
        eadd_done = ctx.enter_context(nc.semaphore("eadd_done"))
        dve_self = ctx.enter_context(nc.semaphore("dve_self"))

        block = ctx.enter_context(nc.Block())

        # pe_abs tick index of the matmul consuming slot s of iteration ip
        def g_slot(ip, s):
            return ip * MM_PER_IP + 1 + s

        # input DMA submissions cost ~600ns each on a queue; split them
        # across the sync and (otherwise idle) gpsimd queues
        # dc -> dma group; even dc on sync, odd on gpsimd so the first
        # chunk's pair is not queued behind anything
        DGRP = [0, 0, 1, 1, 2, 2, 3, 3]
        DGTOT = [64, 64, 64, 64]

        @block.sync
        def _(sync):
            for dc in range(0, NDC2, 2):
                sync.dma_start(
                    out=tw_t[dc][:], in_=Tw[dc * P : (dc + 1) * P, :]
                ).then_inc(dmag[DGRP[dc]], 16)
                sync.dma_start(
                    out=xt_t[dc][:], in_=xT[dc * P : (dc + 1) * P, :]
                ).then_inc(dmag[DGRP[dc]], 16)
            sync.dma_start(out=sel8_t[:], in_=sel8[:, :]).then_inc(dmag[4], 16)
            sync.dma_start(out=selb_t[:], in_=selb[:, :]).then_inc(dmag[4], 16)
            sync.wait_ge(exp_done, NPAIRS)
            sync.dma_start(out=out_d[:, :], in_=osb_t[:]).then_inc(dma_cnt, 16)
            sync.wait_ge(s_copied, 2)
            sync.dma_start(out=tro_d[:, :], in_=tro_t[:]).then_inc(dma_cnt, 16)

        @block.gpsimd
        def _(gp):
            # small consts first: sel1b gates the interleaved S matmuls
            gp.dma_start(out=sel1b_t[:], in_=sel1b[:, :]).then_inc(dmag[4], 16)
            gp.dma_start(out=identw_t[:], in_=identw[:, :]).then_inc(dmag[4], 16)
            for dc in range(1, NDC2, 2):
                gp.dma_start(
                    out=tw_t[dc][:], in_=Tw[dc * P : (dc + 1) * P, :]
                ).then_inc(dmag[DGRP[dc]], 16)
                gp.dma_start(
                    out=xt_t[dc][:], in_=xT[dc * P : (dc + 1) * P, :]
                ).then_inc(dmag[DGRP[dc]], 16)

        @block.tensor
        def _(tensor):
            w = _WaitTracker(tensor)

            # S k-sum for chunk c (1.0 selection), interleaved into phase 1
            # in a dedicated PSUM bank so S is ready right after the last
            # m copy instead of after a separate serial pass
            def s_mm(c):
                w.wait_ge(dmag[4], 16)  # sel1b (gp submits consts first)
                w.wait_ge(m_copied, c + 1)
                mm = nc.tensor.matmul(
                    pss_t[0:OF, 0:W],
                    sel1b_t[:, c * OF : (c + 1) * OF],
                    m_t[c][:, 0:W],
                    start=(c == 0),
                    stop=(c == NCH - 1),
                )
                if c == NCH - 1:
                    mm.then_inc(s_done, 1)

            # phase 1: m^T chunks (fp8 DoubleRow: 2 contraction rows per
            # PE cell, halving the LDWEIGHTS+matmul count)
            for okb in range(NCH):
                ps = ps_t[okb % 2]
                if okb >= 2:
                    w.wait_ge(m_copied, okb - 1)
                for dc in range(NDC2):
                    w.wait_ge(dmag[DGRP[dc]], DGTOT[DGRP[dc]])
                    mm = nc.tensor.matmul(
                        ps[:, 0:W],
                        tw_t[dc][:, :, okb * P : (okb + 1) * P],
                        xt_t[dc][:, :, 0:W],
                        start=(dc == 0),
                        stop=(dc == NDC2 - 1),
                        perf_mode=mybir.MatmulPerfMode.DoubleRow,
                    )
                    if dc == NDC2 - 1:
                        mm.then_inc(mm_done, 1)
                    if dc == 3 and okb >= 1:
                        s_mm(okb - 1)
            s_mm(NCH - 1)
            # phase 2: pairwise D accumulation, halves interleaved so the
            # two PE column groups (PSUM partitions 0-63 / 64-127) overlap
            for ip in range(NPAIRS):
                dp = dp_t[ip % NDP]
                if ip >= NDP:
                    w.wait_ge(exp_done, ip - NDP + 1)
                if ip == 0:
                    w.wait_ge(s_copied, 1)
                    w.wait_ge(dmag[4], 64)  # identw/sel8/selb
                # one correction matmul covers both halves: nsful rows 0-63
                # hold -S for the first half's window, rows 64-127 for the
                # second (shifted) half's window
                nc.tensor.matmul(
                    dp[:, 0:WT],
                    identw_t[:],
                    nsful_t[:],
                    start=True,
                    stop=False,
                ).then_inc(pe_abs, 1)
                for s, (c, half) in enumerate(slots):
                    po = OF * half
                    if s in a_pos:
                        w.wait_ge(act_abs, ip * SA + a_pos[s] + 1)
                        at = abs8_t[(ip * SA + a_pos[s]) % NB8]
                        st = sel8_t
                    else:
                        w.wait_ge(dve_abs, ip * SD + d_pos[s] + 1)
                        at = absb_t[(ip * SD + d_pos[s]) % NBB]
                        st = selb_t
                    nc.tensor.matmul(
                        dp[po : po + OF, 0:WT],
                        st[:, c * OF : (c + 1) * OF],
                        at[:],
                        start=False,
                        stop=(s >= NSLOT - 2),
                    ).then_inc(pe_abs, 1)

        @block.vector
        def _(vector):
            w = _WaitTracker(vector)
            ds = 0
            nc.vector.memset(eacc_t[0][:], 0.0)
            # phase 1: copy m from PSUM; mon = -m(own cols) from the bf16
            # copy (exactness of the D diagonal requires bias == -bf16(m))
            for okb in range(NCH):
                w.wait_ge(mm_done, okb + 1)
                nc.vector.tensor_copy(m_t[okb][:, 0:W], ps_t[okb % 2][:]).then_inc(
                    dve_self, 1
                )
                ds += 1
                w.wait_ge(dve_self, ds)
                nc.vector.tensor_scalar_mul(
                    mon_t[okb][:], m_t[okb][:, 0:ROWS], -1.0
                ).then_inc(m_copied, 1)
            # S tiles: -S in bf16 per half window (rows 0-63: cols 0..288,
            # rows 64-127: cols 32..320), with ln2 added on each half's
            # distance-8 block (last 32 cols) so the doubly-covered pairs
            # contribute exactly half from each core; exp bias columns from
            # the bf16 round-trip
            w.wait_ge(s_done, 1)
            nc.vector.tensor_scalar_mul(
                nsful_t[0:OF, 0:TRW], pss_t[0:OF, 0:TRW], -1.0
            )
            nc.vector.tensor_scalar(
                nsful_t[0:OF, TRW:WT],
                pss_t[0:OF, TRW:WT],
                -1.0,
                -LN2,
                mybir.AluOpType.mult,
                mybir.AluOpType.add,
            )
            nc.vector.tensor_scalar_mul(
                nsful_t[OF:P, 0:TRW], pss_t[0:OF, HS : HS + TRW], -1.0
            )
            nc.vector.tensor_scalar(
                nsful_t[OF:P, TRW:WT],
                pss_t[0:OF, WT:W],
                -1.0,
                -LN2,
                mybir.AluOpType.mult,
                mybir.AluOpType.add,
            )
            nc.vector.tensor_copy(stmp_t[:], pss_t[0:OF, 0:ROWS]).then_inc(
                dve_self, 1
            )
            ds += 1
            w.wait_ge(dve_self, ds)
            nc.vector.tensor_scalar_mul(
                sbias_t[0:OF, :], stmp_t[:, 0:NPAIRS], -1.0
            )
            nc.vector.tensor_scalar_mul(
                sbias_t[OF:P, :], stmp_t[:, NPAIRS:ROWS], -1.0
            ).then_inc(s_copied, 1)
            # phase 2: relu tiles via 2-scalar tensor_scalar (add, max):
            # relu(m_j - m_i) = (m_j + (-m_i)) max 0 — 2x DVE perf mode
            ALU = mybir.AluOpType

            def emit_eadd(j):
                w.wait_ge(exp_done, j + 1)
                nc.vector.tensor_tensor(
                    eacc_t[(j + 1) % 2][:],
                    eacc_t[j % 2][:],
                    esc_t[j % 2][:, HS:WT],
                    ALU.add,
                ).then_inc(eadd_done, 1)

            for ip in range(NPAIRS):
                qlast = (ip + 1) * SD - 1
                if qlast >= NBB:
                    qo = qlast - NBB
                    w.wait_ge(pe_abs, g_slot(qo // SD, dve_idx[qo % SD]) + 1)
                for n, s in enumerate(dve_idx):
                    c, half = slots[s]
                    il = half * NPAIRS + ip
                    w.wait_ge(m_copied, c + 1)
                    nc.vector.tensor_scalar(
                        absb_t[(ip * SD + n) % NBB][:],
                        m_t[c][:, HS * half : HS * half + WT],
                        mon_t[c][:, il : il + 1],
                        0.0,
                        ALU.add,
                        ALU.max,
                    ).then_inc(dve_abs, 1)
                if ip >= EDA:
                    emit_eadd(ip - EDA)
            for j in range(NPAIRS - EDA, NPAIRS):
                emit_eadd(j)
            # transpose partials out (the wait also orders the same-engine
            # read of eacc after the final E-add's write)
            w.wait_ge(eadd_done, NPAIRS)
            nc.vector.tensor_copy(tro_t[:], eacc_t[0][:]).then_inc(s_copied, 1)

        @block.scalar
        def _(scalar):
            # Software-pipelined: the exp for ip is emitted after the relu
            # tiles of ip+ED so the in-order ACT engine never stalls tile
            # production on the cross-engine exp dependency chain.
            w = _WaitTracker(scalar)
            AF = mybir.ActivationFunctionType

            def emit_exp(ip):
                w.wait_ge(s_copied, 1)
                w.wait_ge(pe_abs, (ip + 1) * MM_PER_IP)
                if ip >= 2:
                    w.wait_ge(exp_done, ip - 1)  # esc ping-pong WAW
                    w.wait_ge(eadd_done, ip - 1)  # esc consumed by E-add
                nc.scalar.activation(
                    esc_t[ip % 2][:],
                    dp_t[ip % NDP][:],
                    AF.Exp,
                    bias=sbias_t[:, ip : ip + 1],
                    scale=-1.0,
                    accum_out=osb_t[:, ip : ip + 1],
                ).then_inc(exp_done, 1)

            for ip in range(NPAIRS):
                nlast = (ip + 1) * SA - 1
                if nlast >= NB8:
                    no = nlast - NB8
                    w.wait_ge(pe_abs, g_slot(no // SA, act_idx[no % SA]) + 1)
                for n, s in enumerate(act_idx):
                    c, half = slots[s]
                    il = half * NPAIRS + ip
                    w.wait_ge(m_copied, c + 1)
                    nc.scalar.activation(
                        abs8_t[(ip * SA + n) % NB8][:],
                        m_t[c][:, HS * half : HS * half + WT],
                        AF.Relu,
                        bias=mon_t[c][:, il : il + 1],
                        scale=1.0,
                    ).then_inc(act_abs, 1)
                if ip >= ED:
                    emit_exp(ip - ED)
            for j in range(NPAIRS - ED, NPAIRS):
                emit_exp(j)

    return nc


def _get_nc():
    if "nc" not in _cached:
        _cached["nc"] = _build_nc()
    return _cached["nc"]


def _sel_consts():
    # sel[:, c*64:(c+1)*64][p, o] = v iff o == 8*c + p//16: chunk c's
    # partition (o', k) contributes to output row 8c + o'.  Weight 2.0
    # for the relu sums, 1.0 for the plain S k-sums; ident adds the
    # -S_j correction; fold sums the two partition halves.
    sel = np.zeros((P, NCH * OF), np.float32)
    for c in range(NCH):
        for p in range(P):
            sel[p, c * OF + 8 * c + p // KD] = 2.0
    identw = np.eye(P, dtype=np.float32)
    return (
        sel.astype(ml_dtypes.float8_e5m2),
        sel.astype(ml_dtypes.bfloat16),
        (sel * 0.5).astype(ml_dtypes.bfloat16),
        identw.astype(ml_dtypes.bfloat16),
    )


def kernel(x, T):
    global last_exec_time_ns
    x = np.ascontiguousarray(np.asarray(x, dtype=np.float32))
    T = np.ascontiguousarray(np.asarray(T, dtype=np.float32))
    assert x.shape == (B, DIM) and T.shape == (DIM, OK)

    nc = _get_nc()
    sel8_np, selb_np, sel1b_np, identw_np = _sel_consts()
    # DoubleRow interleave: dram row (dcp*128+p) = contraction rows
    # (dcp*256+2p, dcp*256+2p+1) concatenated
    T_f8 = np.ascontiguousarray(
        T.astype(ml_dtypes.float8_e5m2).reshape(NDC2 * P, 2 * OK)
    )

    in_maps = []
    for c in range(NCORES):
        idx = (c * ROWS + np.arange(W)) % B
        xT_c = np.ascontiguousarray(
            x[idx].T.astype(ml_dtypes.float8_e5m2).reshape(NDC2 * P, 2 * W)
        )
        in_maps.append(
            {
                "xT": xT_c,
                "Tw": T_f8,
                "sel8": sel8_np,
                "selb": selb_np,
                "sel1b": sel1b_np,
                "identw": identw_np,
            }
        )

    trace = os.environ.get("KERNEL_TRACE") == "1"
    if trace:
        trace = _install_ntff_hook()
        tmpdir = os.environ.get("KERNEL_TRACE_DIR") or None
        if tmpdir:
            os.makedirs(tmpdir, exist_ok=True)
    else:
        tmpdir = None
    res = run_bass_kernel_spmd(
        nc, in_maps, core_ids=list(range(NCORES)), trace=trace, tmpdir=tmpdir
    )
    last_exec_time_ns = res.exec_time_ns

    out_full = np.zeros((B, OF), np.float64)
    for c in range(NCORES):
        r = np.asarray(res.results[c]["out"], dtype=np.float64)  # [128, 32]
        blk = out_full[c * ROWS : (c + 1) * ROWS]
        blk[0:NPAIRS] += r[:OF].T
        blk[NPAIRS:ROWS] += r[OF:].T
        tr = np.asarray(res.results[c]["tro"], dtype=np.float64)  # [128, 256]
        rows0 = (c * ROWS + HS + np.arange(TRW)) % B  # first half: cols 32..288
        rows1 = (c * ROWS + 2 * HS + np.arange(TRW)) % B  # second: cols 64..320
        np.add.at(out_full, rows0, tr[:OF].T)
        np.add.at(out_full, rows1, tr[OF:].T)
    return np.concatenate([x, out_full.astype(np.float32)], axis=1)


# revision 51
# speedup vs baseline: 1.0059x; 1.0059x over previous
"""Trainium2 Bass kernel for nn_MinibatchDiscrimination (v2, symmetric).

Reference math:
    m = (x @ T).reshape(B, 64, 16)                      # B=512
    D[i, j, o] = sum_k |m[i,o,k] - m[j,o,k]|
    out[i, o] = sum_j exp(-D[i,j,o])
    return concat([x, out], axis=1)                     # [512, 2112]

Device strategy (8 NeuronCores):
  exp(-D) is symmetric in (i, j), so each unordered pair is evaluated
  once and contributes to BOTH out[i] and out[j].  The batch is split
  into sixteen 32-row blocks; core c receives x^T with rows rotated so
  its two own blocks (rows 64c..64c+63) sit at local columns 0..63.
  Each own row computes D only against a WT=288-column window of 9
  consecutive 32-blocks starting at its own block: rows 0..31 use local
  columns 0..288, rows 32..63 use 32..320 (so m is needed for local
  columns 0..320 only).  Block-distance 1..7 pairs are covered by
  exactly one block's window; distance-8 pairs are covered from both
  sides, made exact by adding ln(2) to D on each window's last 32
  columns so the two cores contribute exactly half of exp(-D) each.
  Per own row the device produces the window row-sum (accum_out of the
  Exp activation); the cross contributions sum_{i} exp(-D[i, j]) for
  the window minus the own block are accumulated on DVE in bf16 (all
  those values underflow anyway) and added to the other rows' outputs
  on the host, which sums the per-core partial outputs.

  On-device pipeline per core: m^T in layout [(o,k) chunks of 128, 320]
  via fp8 DoubleRow matmuls (2 contraction rows per PE cell, host
  pre-interleaves x^T and T); per own row i the L1 distance uses the
  relu decomposition sum_k |d_k| = S_i - S_j + 2*sum_k relu(m_j - m_i)
  (no fused |a-b| op on this walrus), with relu tiles produced by
  ScalarE (Relu activation, fp8 out) and VectorE (2-scalar tensor_scalar
  (add, max) — 2x DVE perf mode; scalar_tensor_tensor is 1x-only) on a
  per-(chunk, half) slot split; TensorE reduces over k with one-hot
  2.0-selection matmuls into PSUM, two rows (i, i+32) interleaved in the
  two PSUM partition halves so their matmuls overlap in distinct PE
  column groups.  One Exp activation per row pair (scale=-1, bias
  -bf16(S_i)) emits the row sums via accum_out.  S k-sums run in a
  dedicated PSUM bank interleaved into phase 1; input DMA submissions
  are split across the sync and gpsimd queues.
  The diagonal D_ii is exactly 0: relu terms vanish identically (both
  operands read the same bf16 values) and the -bf16(S_j) correction
  cancels the -bf16(S_i) exp bias exactly.
  Raw bass (explicit engine blocks + standalone semaphore waits): the
  walrus rejects instructions with >1 inline sync-wait.
"""

import math
import os
import sys
from contextlib import ExitStack

import numpy as np

sys.path.insert(0, "/opt/trn_rl_repo")

import concourse.bass as bass  # noqa: E402
import concourse.mybir as mybir  # noqa: E402
from concourse.bass_utils import run_bass_kernel_spmd  # noqa: E402

import ml_dtypes  # noqa: E402

P = 128
B = 512
DIM = 2048
OF = 64  # out features
KD = 16  # kernel dim
OK = OF * KD  # 1024
NCORES = 8
ROWS = B // NCORES  # 64 own rows per core
W = 320  # m columns (union of the two halves' windows)
WT = 288  # per-half window width (own 32-block + 8 more 32-blocks)
HS = 32  # window shift of the second half (rows 32..63)
TRW = 256  # transpose-partial columns (window minus own 32-block)
NCH = OK // P  # 8 (o,k)-chunks
NDC2 = DIM // (2 * P)  # 8 DoubleRow contraction chunks (256 rows each)
NPAIRS = ROWS // 2  # 32 ip iterations (2 rows per ip)
NSLOT = 2 * NCH  # 16 (chunk, half) slots per ip

ACT_SLOTS = int(os.environ.get("KERNEL_ACT_SLOTS", "5"))  # slots on ScalarE
NB8 = int(os.environ.get("KERNEL_NB8", "15"))  # fp8 abs tile ring size
NBB = int(os.environ.get("KERNEL_NBB", "33"))  # bf16 abs tile ring size
ED = int(os.environ.get("KERNEL_ED", "2"))  # exp emitted ED ips late
EDA = ED + 1  # E-accumulate emitted EDA ips late
NDP = 4

BF16 = mybir.dt.bfloat16
F32 = mybir.dt.float32
FP8 = mybir.dt.float8e5  # e5m2

LN2 = math.log(2.0)

last_exec_time_ns = None

_cached = {}


def _install_ntff_hook():
    """The agent image's `antenv` lacks `axon_hooks`; recreate the NTFF
    profile hook via ctypes against libaxon_pjrt.so and keep artifacts
    local."""
    import contextlib
    import ctypes
    import types

    try:
        import antenv.axon_hooks  # noqa: F401

        return True
    except ImportError:
        pass

    so_path = "/opt/axon/libaxon_pjrt.so"
    if not os.path.exists(so_path):
        return False
    lib = ctypes.CDLL(so_path)
    if not hasattr(lib, "axon_start_nrt_profile"):
        return False
    lib.axon_start_nrt_profile.argtypes = [
        ctypes.POINTER(ctypes.c_int64),
        ctypes.c_size_t,
    ]
    lib.axon_start_nrt_profile.restype = ctypes.c_int64
    lib.axon_stop_nrt_profile.argtypes = [ctypes.c_char_p]
    lib.axon_stop_nrt_profile.restype = ctypes.c_int64

    @contextlib.contextmanager
    def _hook(output_dir, device_ids):
        import jax

        jax.devices()
        if device_ids:
            ids = (ctypes.c_int64 * len(device_ids))(*device_ids)
            rc = lib.axon_start_nrt_profile(ids, len(device_ids))
        else:
            rc = lib.axon_start_nrt_profile(None, 0)
        if rc != 0:
            raise RuntimeError(f"axon_start_nrt_profile rc={rc}")
        try:
            yield
        finally:
            n = lib.axon_stop_nrt_profile(str(output_dir).encode())
            print(f"ntff profile: {n} file(s) written to {output_dir}", file=sys.stderr)

    mod = types.ModuleType("antenv.axon_hooks")
    _state = {"hook": _hook}
    mod.set_axon_ntff_profile_hook = lambda h: _state.__setitem__("hook", h)
    mod.get_axon_ntff_profile_hook = lambda: _state["hook"]
    import antenv

    sys.modules["antenv.axon_hooks"] = mod
    antenv.axon_hooks = mod

    import concourse.bass_utils as bu

    bu.upload_artifacts = lambda tmpdir: str(tmpdir)
    return True


class _WaitTracker:
    """Emit a standalone wait only when this engine hasn't already
    waited for (at least) the needed value on that semaphore."""

    def __init__(self, eng):
        self.eng = eng
        self.seen = {}

    def wait_ge(self, sem, val):
        if self.seen.get(sem.num, -1) >= val:
            return
        self.eng.wait_ge(sem, val)
        self.seen[sem.num] = val


MM_PER_IP = 1 + NSLOT  # 1 correction (both halves) + 16 slot matmuls


def _slot_layout(act_slots=ACT_SLOTS):
    """slot s = (chunk, half); choose which slots run on ScalarE
    (spread across the slot sequence), the rest on VectorE."""
    slots = [(c, h) for c in range(NCH) for h in range(2)]
    act_idx = sorted({round(i * (NSLOT - 1) / max(act_slots - 1, 1)) for i in range(act_slots)}) if act_slots else []
    # ensure exactly act_slots distinct indices
    i = 0
    while len(act_idx) < act_slots:
        if i not in act_idx:
            act_idx.append(i)
        i += 1
    act_idx = sorted(act_idx[:act_slots])
    dve_idx = [s for s in range(NSLOT) if s not in act_idx]
    return slots, act_idx, dve_idx


def _build_nc(act_slots=ACT_SLOTS):
    nc = bass.Bass()
    AF = mybir.ActivationFunctionType
    ALU = mybir.AluOpType

    slots, act_idx, dve_idx = _slot_layout(act_slots)
    SA = len(act_idx)
    SD = len(dve_idx)
    assert SD >= 8, "E-add same-engine spacing relies on >=8 DVE ops per ip"
    a_pos = {s: n for n, s in enumerate(act_idx)}  # slot -> per-ip act index
    d_pos = {s: n for n, s in enumerate(dve_idx)}

    # phase-1 inputs in DoubleRow interleave: row (dcp*128+p) holds the
    # two contraction rows (dcp*256+2p, dcp*256+2p+1) concatenated
    xT = nc.declare_dram_parameter("xT", [NDC2 * P, 2 * W], FP8, isOutput=False)
    Tw = nc.declare_dram_parameter("Tw", [NDC2 * P, 2 * OK], FP8, isOutput=False)
    sel8 = nc.declare_dram_parameter("sel8", [P, NCH * OF], FP8, isOutput=False)
    selb = nc.declare_dram_parameter("selb", [P, NCH * OF], BF16, isOutput=False)
    sel1b = nc.declare_dram_parameter("sel1b", [P, NCH * OF], BF16, isOutput=False)
    identw = nc.declare_dram_parameter("identw", [P, P], BF16, isOutput=False)
    out_d = nc.declare_dram_parameter("out", [P, NPAIRS], F32, isOutput=True)
    tro_d = nc.declare_dram_parameter("tro", [P, TRW], F32, isOutput=True)

    ctx = ExitStack()
    with ctx:
        tw_t = [ctx.enter_context(nc.sbuf_tensor(f"tw{i}", [P, 2, OK], FP8)) for i in range(NDC2)]
        xt_t = [ctx.enter_context(nc.sbuf_tensor(f"xt{i}", [P, 2, W], FP8)) for i in range(NDC2)]
        m_t = [ctx.enter_context(nc.sbuf_tensor(f"m{i}", [P, W], BF16)) for i in range(NCH)]
        mon_t = [ctx.enter_context(nc.sbuf_tensor(f"mon{i}", [P, ROWS], F32)) for i in range(NCH)]
        sel8_t = ctx.enter_context(nc.sbuf_tensor("sel8t", [P, NCH * OF], FP8))
        selb_t = ctx.enter_context(nc.sbuf_tensor("selbt", [P, NCH * OF], BF16))
        sel1b_t = ctx.enter_context(nc.sbuf_tensor("sel1bt", [P, NCH * OF], BF16))
        identw_t = ctx.enter_context(nc.sbuf_tensor("identwt", [P, P], BF16))
        abs8_t = [ctx.enter_context(nc.sbuf_tensor(f"abs8_{i}", [P, WT], FP8)) for i in range(NB8)]
        absb_t = [ctx.enter_context(nc.sbuf_tensor(f"absb_{i}", [P, WT], BF16)) for i in range(NBB)]
        nsful_t = ctx.enter_context(nc.sbuf_tensor("nsful", [P, WT], BF16))
        sbias_t = ctx.enter_context(nc.sbuf_tensor("sbias", [P, NPAIRS], F32))
        stmp_t = ctx.enter_context(nc.sbuf_tensor("stmp", [OF, ROWS], BF16))
        esc_t = [ctx.enter_context(nc.sbuf_tensor(f"esct{i}", [P, WT], BF16)) for i in range(2)]
        eacc_t = [ctx.enter_context(nc.sbuf_tensor(f"eacc{i}", [P, TRW], BF16)) for i in range(2)]
        osb_t = ctx.enter_context(nc.sbuf_tensor("osbt", [P, NPAIRS], F32))
        tro_t = ctx.enter_context(nc.sbuf_tensor("trot", [P, TRW], F32))

        ps_t = [ctx.enter_context(nc.psum_tensor(f"ps{i}", [P, W], F32)) for i in range(2)]
        dp_t = [ctx.enter_context(nc.psum_tensor(f"dp{i}", [P, WT], F32)) for i in range(NDP)]
        pss_t = ctx.enter_context(nc.psum_tensor("pss", [OF, W], F32))

        dmag = [ctx.enter_context(nc.semaphore(f"dmag{i}")) for i in range(5)]
        dma_cnt = ctx.enter_context(nc.semaphore("dma_cnt"))
        mm_done = ctx.enter_context(nc.semaphore("mm_done"))
        m_copied = ctx.enter_context(nc.semaphore("m_copied"))
        s_done = ctx.enter_context(nc.semaphore("s_done"))
        s_copied = ctx.enter_context(nc.semaphore("s_copied"))
        pe_abs = ctx.enter_context(nc.semaphore("pe_abs"))
        act_abs = ctx.enter_context(nc.semaphore("act_abs"))
        dve_abs = ctx.enter_context(nc.semaphore("dve_abs"))
        exp_done = ctx.enter_context(nc.semaphore("exp_done"))
        eadd_done = ctx.enter_context(nc.semaphore("eadd_done"))
        dve_self = ctx.enter_context(nc.semaphore("dve_self"))

        block = ctx.enter_context(nc.Block())

        # pe_abs tick index of the matmul consuming slot s of iteration ip
        def g_slot(ip, s):
            return ip * MM_PER_IP + 1 + s

        # input DMA submissions cost ~600ns each on a queue; split them
        # across the sync and (otherwise idle) gpsimd queues
        # dc -> dma group; even dc on sync, odd on gpsimd so the first
        # chunk's pair is not queued behind anything
        DGRP = [0, 0, 1, 1, 2, 2, 3, 3]
        DGTOT = [64, 64, 64, 64]

        @block.sync
        def _(sync):
            for dc in range(0, NDC2, 2):
                sync.dma_start(
                    out=tw_t[dc][:], in_=Tw[dc * P : (dc + 1) * P, :]
                ).then_inc(dmag[DGRP[dc]], 16)
                sync.dma_start(
                    out=xt_t[dc][:], in_=xT[dc * P : (dc + 1) * P, :]
                ).then_inc(dmag[DGRP[dc]], 16)
            sync.dma_start(out=sel8_t[:], in_=sel8[:, :]).then_inc(dmag[4], 16)
            sync.dma_start(out=selb_t[:], in_=selb[:, :]).then_inc(dmag[4], 16)
            sync.wait_ge(exp_done, NPAIRS)
            sync.dma_start(out=out_d[:, :], in_=osb_t[:]).then_inc(dma_cnt, 16)
            sync.wait_ge(s_copied, 2)
            sync.dma_start(out=tro_d[:, :], in_=tro_t[:]).then_inc(dma_cnt, 16)

        @block.gpsimd
        def _(gp):
            # small consts first: sel1b gates the interleaved S matmuls
            gp.dma_start(out=sel1b_t[:], in_=sel1b[:, :]).then_inc(dmag[4], 16)
            gp.dma_start(out=identw_t[:], in_=identw[:, :]).then_inc(dmag[4], 16)
            for dc in range(1, NDC2, 2):
                gp.dma_start(
                    out=tw_t[dc][:], in_=Tw[dc * P : (dc + 1) * P, :]
                ).then_inc(dmag[DGRP[dc]], 16)
                gp.dma_start(
                    out=xt_t[dc][:], in_=xT[dc * P : (dc + 1) * P, :]
                ).then_inc(dmag[DGRP[dc]], 16)

        @block.tensor
        def _(tensor):
            w = _WaitTracker(tensor)

            # S k-sum for chunk c (1.0 selection), interleaved into phase 1
            # in a dedicated PSUM bank so S is ready right after the last
            # m copy instead of after a separate serial pass
            def s_mm(c):
                w.wait_ge(dmag[4], 16)  # sel1b (gp submits consts first)
                w.wait_ge(m_copied, c + 1)
                mm = nc.tensor.matmul(
                    pss_t[0:OF, 0:W],
                    sel1b_t[:, c * OF : (c + 1) * OF],
                    m_t[c][:, 0:W],
                    start=(c == 0),
                    stop=(c == NCH - 1),
                )
                if c == NCH - 1:
                    mm.then_inc(s_done, 1)

            # phase 1: m^T chunks (fp8 DoubleRow: 2 contraction rows per
            # PE cell, halving the LDWEIGHTS+matmul count)
            for okb in range(NCH):
                ps = ps_t[okb % 2]
                if okb >= 2:
                    w.wait_ge(m_copied, okb - 1)
                for dc in range(NDC2):
                    w.wait_ge(dmag[DGRP[dc]], DGTOT[DGRP[dc]])
                    mm = nc.tensor.matmul(
                        ps[:, 0:W],
                        tw_t[dc][:, :, okb * P : (okb + 1) * P],
                        xt_t[dc][:, :, 0:W],
                        start=(dc == 0),
                        stop=(dc == NDC2 - 1),
                        perf_mode=mybir.MatmulPerfMode.DoubleRow,
                    )
                    if dc == NDC2 - 1:
                        mm.then_inc(mm_done, 1)
                    if dc == 3 and okb >= 1:
                        s_mm(okb - 1)
            s_mm(NCH - 1)
            # phase 2: pairwise D accumulation, halves interleaved so the
            # two PE column groups (PSUM partitions 0-63 / 64-127) overlap
            for ip in range(NPAIRS):
                dp = dp_t[ip % NDP]
                if ip >= NDP:
                    w.wait_ge(exp_done, ip - NDP + 1)
                if ip == 0:
                    w.wait_ge(s_copied, 1)
                    w.wait_ge(dmag[4], 64)  # identw/sel8/selb
                # one correction matmul covers both halves: nsful rows 0-63
                # hold -S for the first half's window, rows 64-127 for the
                # second (shifted) half's window
                nc.tensor.matmul(
                    dp[:, 0:WT],
                    identw_t[:],
                    nsful_t[:],
                    start=True,
                    stop=False,
                ).then_inc(pe_abs, 1)
                for s, (c, half) in enumerate(slots):
                    po = OF * half
                    if s in a_pos:
                        w.wait_ge(act_abs, ip * SA + a_pos[s] + 1)
                        at = abs8_t[(ip * SA + a_pos[s]) % NB8]
                        st = sel8_t
                    else:
                        w.wait_ge(dve_abs, ip * SD + d_pos[s] + 1)
                        at = absb_t[(ip * SD + d_pos[s]) % NBB]
                        st = selb_t
                    nc.tensor.matmul(
                        dp[po : po + OF, 0:WT],
                        st[:, c * OF : (c + 1) * OF],
                        at[:],
                        start=False,
                        stop=(s >= NSLOT - 2),
                    ).then_inc(pe_abs, 1)

        @block.vector
        def _(vector):
            w = _WaitTracker(vector)
            ds = 0
            nc.vector.memset(eacc_t[0][:], 0.0)
            # phase 1: copy m from PSUM; mon = -m(own cols) from the bf16
            # copy (exactness of the D diagonal requires bias == -bf16(m))
            for okb in range(NCH):
                w.wait_ge(mm_done, okb + 1)
                nc.vector.tensor_copy(m_t[okb][:, 0:W], ps_t[okb % 2][:]).then_inc(
                    dve_self, 1
                )
                ds += 1
                w.wait_ge(dve_self, ds)
                nc.vector.tensor_scalar_mul(
                    mon_t[okb][:], m_t[okb][:, 0:ROWS], -1.0
                ).then_inc(m_copied, 1)
            # S tiles: -S in bf16 per half window (rows 0-63: cols 0..288,
            # rows 64-127: cols 32..320), with ln2 added on each half's
            # distance-8 block (last 32 cols) so the doubly-covered pairs
            # contribute exactly half from each core; exp bias columns from
            # the bf16 round-trip
            w.wait_ge(s_done, 1)
            nc.vector.tensor_scalar_mul(
                nsful_t[0:OF, 0:TRW], pss_t[0:OF, 0:TRW], -1.0
            )
            nc.vector.tensor_scalar(
                nsful_t[0:OF, TRW:WT],
                pss_t[0:OF, TRW:WT],
                -1.0,
                -LN2,
                mybir.AluOpType.mult,
                mybir.AluOpType.add,
            )
            nc.vector.tensor_scalar_mul(
                nsful_t[OF:P, 0:TRW], pss_t[0:OF, HS : HS + TRW], -1.0
            )
            nc.vector.tensor_scalar(
                nsful_t[OF:P, TRW:WT],
                pss_t[0:OF, WT:W],
                -1.0,
                -LN2,
                mybir.AluOpType.mult,
                mybir.AluOpType.add,
            )
            nc.vector.tensor_copy(stmp_t[:], pss_t[0:OF, 0:ROWS]).then_inc(
                dve_self, 1
            )
            ds += 1
            w.wait_ge(dve_self, ds)
            nc.vector.tensor_scalar_mul(
                sbias_t[0:OF, :], stmp_t[:, 0:NPAIRS], -1.0
            )
            nc.vector.tensor_scalar_mul(
                sbias_t[OF:P, :], stmp_t[:, NPAIRS:ROWS], -1.0
            ).then_inc(s_copied, 1)
            # phase 2: relu tiles via 2-scalar tensor_scalar (add, max):
            # relu(m_j - m_i) = (m_j + (-m_i)) max 0 — 2x DVE perf mode
            ALU = mybir.AluOpType

            def emit_eadd(j):
                w.wait_ge(exp_done, j + 1)
                nc.vector.tensor_tensor(
                    eacc_t[(j + 1) % 2][:],
                    eacc_t[j % 2][:],
                    esc_t[j % 2][:, HS:WT],
                    ALU.add,
                ).then_inc(eadd_done, 1)

            for ip in range(NPAIRS):
                qlast = (ip + 1) * SD - 1
                if qlast >= NBB:
                    qo = qlast - NBB
                    w.wait_ge(pe_abs, g_slot(qo // SD, dve_idx[qo % SD]) + 1)
                for n, s in enumerate(dve_idx):
                    c, half = slots[s]
                    il = half * NPAIRS + ip
                    w.wait_ge(m_copied, c + 1)
                    nc.vector.tensor_scalar(
                        absb_t[(ip * SD + n) % NBB][:],
                        m_t[c][:, HS * half : HS * half + WT],
                        mon_t[c][:, il : il + 1],
                        0.0,
                        ALU.add,
                        ALU.max,
                    ).then_inc(dve_abs, 1)
                if ip >= EDA:
                    emit_eadd(ip - EDA)
            for j in range(NPAIRS - EDA, NPAIRS):
                emit_eadd(j)
            # transpose partials out (the wait also orders the same-engine
            # read of eacc after the final E-add's write)
            w.wait_ge(eadd_done, NPAIRS)
            nc.vector.tensor_copy(tro_t[:], eacc_t[0][:]).then_inc(s_copied, 1)

        @block.scalar
        def _(scalar):
            # Software-pipelined: the exp for ip is emitted after the relu
            # tiles of ip+ED so the in-order ACT engine never stalls tile
            # production on the cross-engine exp dependency chain.
            w = _WaitTracker(scalar)
            AF = mybir.ActivationFunctionType

            def emit_exp(ip):
                w.wait_ge(s_copied, 1)
                w.wait_ge(pe_abs, (ip + 1) * MM_PER_IP)
                if ip >= 2:
                    w.wait_ge(exp_done, ip - 1)  # esc ping-pong WAW
                    w.wait_ge(eadd_done, ip - 1)  # esc consumed by E-add
                nc.scalar.activation(
                    esc_t[ip % 2][:],
                    dp_t[ip % NDP][:],
                    AF.Exp,
                    bias=sbias_t[:, ip : ip + 1],
                    scale=-1.0,
                    accum_out=osb_t[:, ip : ip + 1],
                ).then_inc(exp_done, 1)

            for ip in range(NPAIRS):
                nlast = (ip + 1) * SA - 1
                if nlast >= NB8:
                    no = nlast - NB8
                    w.wait_ge(pe_abs, g_slot(no // SA, act_idx[no % SA]) + 1)
                for n, s in enumerate(act_idx):
                    c, half = slots[s]
                    il = half * NPAIRS + ip
                    w.wait_ge(m_copied, c + 1)
                    nc.scalar.activation(
                        abs8_t[(ip * SA + n) % NB8][:],
                        m_t[c][:, HS * half : HS * half + WT],
                        AF.Relu,
                        bias=mon_t[c][:, il : il + 1],
                        scale=1.0,
                    ).then_inc(act_abs, 1)
                if ip >= ED:
                    emit_exp(ip - ED)
            for j in range(NPAIRS - ED, NPAIRS):
                emit_exp(j)

    return nc


def _get_nc():
    if "nc" not in _cached:
        _cached["nc"] = _build_nc()
    return _cached["nc"]


def _sel_consts():
    # sel[:, c*64:(c+1)*64][p, o] = v iff o == 8*c + p//16: chunk c's
    # partition (o', k) contributes to output row 8c + o'.  Weight 2.0
    # for the relu sums, 1.0 for the plain S k-sums; ident adds the
    # -S_j correction; fold sums the two partition halves.
    sel = np.zeros((P, NCH * OF), np.float32)
    for c in range(NCH):
        for p in range(P):
            sel[p, c * OF + 8 * c + p // KD] = 2.0
    identw = np.eye(P, dtype=np.float32)
    return (
        sel.astype(ml_dtypes.float8_e5m2),
        sel.astype(ml_dtypes.bfloat16),
        (sel * 0.5).astype(ml_dtypes.bfloat16),
        identw.astype(ml_dtypes.bfloat16),
    )


def kernel(x, T):
    global last_exec_time_ns
    x = np.ascontiguousarray(np.asarray(x, dtype=np.float32))
    T = np.ascontiguousarray(np.asarray(T, dtype=np.float32))
    assert x.shape == (B, DIM) and T.shape == (DIM, OK)

    nc = _get_nc()
    sel8_np, selb_np, sel1b_np, identw_np = _sel_consts()
    # DoubleRow interleave: dram row (dcp*128+p) = contraction rows
    # (dcp*256+2p, dcp*256+2p+1) concatenated
    T_f8 = np.ascontiguousarray(
        T.astype(ml_dtypes.float8_e5m2).reshape(NDC2 * P, 2 * OK)
    )

    in_maps = []
    for c in range(NCORES):
        idx = (c * ROWS + np.arange(W)) % B
        xT_c = np.ascontiguousarray(
            x[idx].T.astype(ml_dtypes.float8_e5m2).reshape(NDC2 * P, 2 * W)
        )
        in_maps.append(
            {
                "xT": xT_c,
                "Tw": T_f8,
                "sel8": sel8_np,
                "selb": selb_np,
                "sel1b": sel1b_np,
                "identw": identw_np,
            }
        )

    trace = os.environ.get("KERNEL_TRACE") == "1"
    if trace:
        trace = _install_ntff_hook()
        tmpdir = os.environ.get("KERNEL_TRACE_DIR") or None
        if tmpdir:
            os.makedirs(tmpdir, exist_ok=True)
    else:
        tmpdir = None
    res = run_bass_kernel_spmd(
        nc, in_maps, core_ids=list(range(NCORES)), trace=trace, tmpdir=tmpdir
    )
    last_exec_time_ns = res.exec_time_ns

    out_full = np.zeros((B, OF), np.float64)
    for c in range(NCORES):
        r = np.asarray(res.results[c]["out"], dtype=np.float64)  # [128, 32]
        blk = out_full[c * ROWS : (c + 1) * ROWS]
        blk[0:NPAIRS] += r[:OF].T
        blk[NPAIRS:ROWS] += r[OF:].T
        tr = np.asarray(res.results[c]["tro"], dtype=np.float64)  # [128, 256]
        rows0 = (c * ROWS + HS + np.arange(TRW)) % B  # first half: cols 32..288
        rows1 = (c * ROWS + 2 * HS + np.arange(TRW)) % B  # second: cols 64..320
        np.add.at(out_full, rows0, tr[:OF].T)
        np.add.at(out_full, rows1, tr[OF:].T)
    return np.concatenate([x, out_full.astype(np.float32)], axis=1)


# revision 56
# speedup vs baseline: 1.0481x; 1.0420x over previous
"""Trainium2 Bass kernel for nn_MinibatchDiscrimination (v2, symmetric).

Reference math:
    m = (x @ T).reshape(B, 64, 16)                      # B=512
    D[i, j, o] = sum_k |m[i,o,k] - m[j,o,k]|
    out[i, o] = sum_j exp(-D[i,j,o])
    return concat([x, out], axis=1)                     # [512, 2112]

Device strategy (8 NeuronCores):
  exp(-D) is symmetric in (i, j), so each unordered pair is evaluated
  once and contributes to BOTH out[i] and out[j].  The batch is split
  into sixteen 32-row blocks; core c receives x^T with rows rotated so
  its two own blocks (rows 64c..64c+63) sit at local columns 0..63.
  Each own row computes D only against a WT=288-column window of 9
  consecutive 32-blocks starting at its own block: rows 0..31 use local
  columns 0..288, rows 32..63 use 32..320 (so m is needed for local
  columns 0..320 only).  Block-distance 1..7 pairs are covered by
  exactly one block's window; distance-8 pairs are covered from both
  sides, made exact by adding ln(2) to D on each window's last 32
  columns so the two cores contribute exactly half of exp(-D) each.
  Per own row the device produces the window row-sum (accum_out of the
  Exp activation); the cross contributions sum_{i} exp(-D[i, j]) for
  the window minus the own block are accumulated on DVE in bf16 (all
  those values underflow anyway) and added to the other rows' outputs
  on the host, which sums the per-core partial outputs.

  On-device pipeline per core: m^T in layout [(o,k) chunks of 128, 320]
  via fp8 DoubleRow matmuls (2 contraction rows per PE cell, host
  pre-interleaves x^T and T); per own row i the L1 distance uses the
  relu decomposition sum_k |d_k| = S_i - S_j + 2*sum_k relu(m_j - m_i)
  (no fused |a-b| op on this walrus), with relu tiles produced by
  ScalarE (Relu activation, fp8 out) and VectorE (2-scalar tensor_scalar
  (add, max) — 2x DVE perf mode; scalar_tensor_tensor is 1x-only) on a
  per-(chunk, half) slot split; TensorE reduces over k with one-hot
  2.0-selection matmuls into PSUM, two rows (i, i+32) interleaved in the
  two PSUM partition halves so their matmuls overlap in distinct PE
  column groups.  One Exp activation per row pair (scale=-1, bias
  -bf16(S_i)) emits the row sums via accum_out.  S k-sums run in a
  dedicated PSUM bank interleaved into phase 1; input DMA submissions
  are split across the sync and gpsimd queues.
  The diagonal D_ii is exactly 0: relu terms vanish identically (both
  operands read the same bf16 values) and the -bf16(S_j) correction
  cancels the -bf16(S_i) exp bias exactly.
  Raw bass (explicit engine blocks + standalone semaphore waits): the
  walrus rejects instructions with >1 inline sync-wait.
"""

import math
import os
import sys
from contextlib import ExitStack

import numpy as np

sys.path.insert(0, "/opt/trn_rl_repo")

import concourse.bass as bass  # noqa: E402
import concourse.mybir as mybir  # noqa: E402
from concourse.bass_utils import run_bass_kernel_spmd  # noqa: E402

import ml_dtypes  # noqa: E402

P = 128
B = 512
DIM = 2048
OF = 64  # out features
KD = 16  # kernel dim
OK = OF * KD  # 1024
NCORES = 8
ROWS = B // NCORES  # 64 own rows per core
W = 320  # m columns (union of the two halves' windows)
WT = 288  # per-half window width (own 32-block + 8 more 32-blocks)
HS = 32  # window shift of the second half (rows 32..63)
TRW = 256  # transpose-partial columns (window minus own 32-block)
NCH = OK // P  # 8 (o,k)-chunks
NDC2 = DIM // (2 * P)  # 8 DoubleRow contraction chunks (256 rows each)
NPAIRS = ROWS // 2  # 32 ip iterations (2 rows per ip)
NSLOT = 2 * NCH  # 16 (chunk, half) slots per ip

ACT_SLOTS = int(os.environ.get("KERNEL_ACT_SLOTS", "5"))  # slots on ScalarE
NB8 = int(os.environ.get("KERNEL_NB8", "15"))  # fp8 abs tile ring size
NBB = int(os.environ.get("KERNEL_NBB", "33"))  # bf16 abs tile ring size
ED = int(os.environ.get("KERNEL_ED", "2"))  # exp emitted ED ips late
EDA = ED + 1  # E-accumulate emitted EDA ips late
NDP = 4

BF16 = mybir.dt.bfloat16
F32 = mybir.dt.float32
FP8 = mybir.dt.float8e5  # e5m2

LN2 = math.log(2.0)

last_exec_time_ns = None

_cached = {}


def _install_ntff_hook():
    """The agent image's `antenv` lacks `axon_hooks`; recreate the NTFF
    profile hook via ctypes against libaxon_pjrt.so and keep artifacts
    local."""
    import contextlib
    import ctypes
    import types

    try:
        import antenv.axon_hooks  # noqa: F401

        return True
    except ImportError:
        pass

    so_path = "/opt/axon/libaxon_pjrt.so"
    if not os.path.exists(so_path):
        return False
    lib = ctypes.CDLL(so_path)
    if not hasattr(lib, "axon_start_nrt_profile"):
        return False
    lib.axon_start_nrt_profile.argtypes = [
        ctypes.POINTER(ctypes.c_int64),
        ctypes.c_size_t,
    ]
    lib.axon_start_nrt_profile.restype = ctypes.c_int64
    lib.axon_stop_nrt_profile.argtypes = [ctypes.c_char_p]
    lib.axon_stop_nrt_profile.restype = ctypes.c_int64

    @contextlib.contextmanager
    def _hook(output_dir, device_ids):
        import jax

        jax.devices()
        if device_ids:
            ids = (ctypes.c_int64 * len(device_ids))(*device_ids)
            rc = lib.axon_start_nrt_profile(ids, len(device_ids))
        else:
            rc = lib.axon_start_nrt_profile(None, 0)
        if rc != 0:
            raise RuntimeError(f"axon_start_nrt_profile rc={rc}")
        try:
            yield
        finally:
            n = lib.axon_stop_nrt_profile(str(output_dir).encode())
            print(f"ntff profile: {n} file(s) written to {output_dir}", file=sys.stderr)

    mod = types.ModuleType("antenv.axon_hooks")
    _state = {"hook": _hook}
    mod.set_axon_ntff_profile_hook = lambda h: _state.__setitem__("hook", h)
    mod.get_axon_ntff_profile_hook = lambda: _state["hook"]
    import antenv

    sys.modules["antenv.axon_hooks"] = mod
    antenv.axon_hooks = mod

    import concourse.bass_utils as bu

    bu.upload_artifacts = lambda tmpdir: str(tmpdir)
    return True


class _WaitTracker:
    """Emit a standalone wait only when this engine hasn't already
    waited for (at least) the needed value on that semaphore."""

    def __init__(self, eng):
        self.eng = eng
        self.seen = {}

    def wait_ge(self, sem, val):
        if self.seen.get(sem.num, -1) >= val:
            return
        self.eng.wait_ge(sem, val)
        self.seen[sem.num] = val


MM_PER_IP = 1 + NSLOT  # 1 correction (both halves) + 16 slot matmuls


def _slot_layout(act_slots=ACT_SLOTS):
    """slot s = (chunk, half); choose which slots run on ScalarE
    (spread across the slot sequence), the rest on VectorE."""
    slots = [(c, h) for c in range(NCH) for h in range(2)]
    act_idx = sorted({round(i * (NSLOT - 1) / max(act_slots - 1, 1)) for i in range(act_slots)}) if act_slots else []
    # ensure exactly act_slots distinct indices
    i = 0
    while len(act_idx) < act_slots:
        if i not in act_idx:
            act_idx.append(i)
        i += 1
    act_idx = sorted(act_idx[:act_slots])
    dve_idx = [s for s in range(NSLOT) if s not in act_idx]
    return slots, act_idx, dve_idx


def _build_nc(act_slots=ACT_SLOTS):
    nc = bass.Bass()
    AF = mybir.ActivationFunctionType
    ALU = mybir.AluOpType

    slots, act_idx, dve_idx = _slot_layout(act_slots)
    SA = len(act_idx)
    SD = len(dve_idx)
    assert SD >= 8, "E-add same-engine spacing relies on >=8 DVE ops per ip"
    a_pos = {s: n for n, s in enumerate(act_idx)}  # slot -> per-ip act index
    d_pos = {s: n for n, s in enumerate(dve_idx)}

    # phase-1 inputs in DoubleRow interleave: row (dcp*128+p) holds the
    # two contraction rows (dcp*256+2p, dcp*256+2p+1) concatenated
    xT = nc.declare_dram_parameter("xT", [NDC2 * P, 2 * W], FP8, isOutput=False)
    Tw = nc.declare_dram_parameter("Tw", [NDC2 * P, 2 * OK], FP8, isOutput=False)
    sel8 = nc.declare_dram_parameter("sel8", [P, NCH * OF], FP8, isOutput=False)
    selb = nc.declare_dram_parameter("selb", [P, NCH * OF], BF16, isOutput=False)
    sel1b = nc.declare_dram_parameter("sel1b", [P, NCH * OF], BF16, isOutput=False)
    identw = nc.declare_dram_parameter("identw", [P, P], BF16, isOutput=False)
    out_d = nc.declare_dram_parameter("out", [P, NPAIRS], F32, isOutput=True)
    tro_d = nc.declare_dram_parameter("tro", [P, TRW], F32, isOutput=True)

    ctx = ExitStack()
    with ctx:
        tw_t = [ctx.enter_context(nc.sbuf_tensor(f"tw{i}", [P, 2, OK], FP8)) for i in range(NDC2)]
        xt_t = [ctx.enter_context(nc.sbuf_tensor(f"xt{i}", [P, 2, W], FP8)) for i in range(NDC2)]
        m_t = [ctx.enter_context(nc.sbuf_tensor(f"m{i}", [P, W], BF16)) for i in range(NCH)]
        mon_t = [ctx.enter_context(nc.sbuf_tensor(f"mon{i}", [P, ROWS], F32)) for i in range(NCH)]
        sel8_t = ctx.enter_context(nc.sbuf_tensor("sel8t", [P, NCH * OF], FP8))
        selb_t = ctx.enter_context(nc.sbuf_tensor("selbt", [P, NCH * OF], BF16))
        sel1b_t = ctx.enter_context(nc.sbuf_tensor("sel1bt", [P, NCH * OF], BF16))
        identw_t = ctx.enter_context(nc.sbuf_tensor("identwt", [P, P], BF16))
        abs8_t = [ctx.enter_context(nc.sbuf_tensor(f"abs8_{i}", [P, WT], FP8)) for i in range(NB8)]
        absb_t = [ctx.enter_context(nc.sbuf_tensor(f"absb_{i}", [P, WT], BF16)) for i in range(NBB)]
        nsful_t = ctx.enter_context(nc.sbuf_tensor("nsful", [P, WT], BF16))
        sbias_t = ctx.enter_context(nc.sbuf_tensor("sbias", [P, NPAIRS], F32))
        stmp_t = ctx.enter_context(nc.sbuf_tensor("stmp", [OF, ROWS], BF16))
        esc_t = [ctx.enter_context(nc.sbuf_tensor(f"esct{i}", [P, WT], BF16)) for i in range(2)]
        eacc_t = [ctx.enter_context(nc.sbuf_tensor(f"eacc{i}", [P, TRW], BF16)) for i in range(2)]
        osb_t = ctx.enter_context(nc.sbuf_tensor("osbt", [P, NPAIRS], F32))
        tro_t = ctx.enter_context(nc.sbuf_tensor("trot", [P, TRW], F32))

        ps_t = [ctx.enter_context(nc.psum_tensor(f"ps{i}", [P, W], F32)) for i in range(3)]
        dp_t = [ctx.enter_context(nc.psum_tensor(f"dp{i}", [P, W], F32)) for i in range(NDP)]
        pss_t = ctx.enter_context(nc.psum_tensor("pss", [P, W], F32))
        # phase-1 m accumulators: one PSUM bank per chunk (dp banks are
        # idle during phase 1), so all chunks finish right after the last
        # input DMA group instead of serially
        mb_t = [ps_t[0], ps_t[1], ps_t[2], dp_t[0], dp_t[1], dp_t[2], dp_t[3], pss_t]

        dmag = [ctx.enter_context(nc.semaphore(f"dmag{i}")) for i in range(5)]
        dma_cnt = ctx.enter_context(nc.semaphore("dma_cnt"))
        mm_done = ctx.enter_context(nc.semaphore("mm_done"))
        m_copied = ctx.enter_context(nc.semaphore("m_copied"))
        s_done = ctx.enter_context(nc.semaphore("s_done"))
        s_copied = ctx.enter_context(nc.semaphore("s_copied"))
        pe_abs = ctx.enter_context(nc.semaphore("pe_abs"))
        act_abs = ctx.enter_context(nc.semaphore("act_abs"))
        dve_abs = ctx.enter_context(nc.semaphore("dve_abs"))
        exp_done = ctx.enter_context(nc.semaphore("exp_done"))
        eadd_done = ctx.enter_context(nc.semaphore("eadd_done"))
        dve_self = ctx.enter_context(nc.semaphore("dve_self"))

        block = ctx.enter_context(nc.Block())

        # pe_abs tick index of the matmul consuming slot s of iteration ip
        def g_slot(ip, s):
            return ip * MM_PER_IP + 1 + s

        # input DMA submissions cost ~600ns each on a queue; split them
        # across the sync and (otherwise idle) gpsimd queues
        # dc -> dma group; even dc on sync, odd on gpsimd so the first
        # chunk's pair is not queued behind anything
        DGRP = [0, 0, 1, 1, 2, 2, 3, 3]
        DGTOT = [64, 64, 64, 64]

        @block.sync
        def _(sync):
            for dc in range(0, NDC2, 2):
                sync.dma_start(
                    out=tw_t[dc][:], in_=Tw[dc * P : (dc + 1) * P, :]
                ).then_inc(dmag[DGRP[dc]], 16)
                sync.dma_start(
                    out=xt_t[dc][:], in_=xT[dc * P : (dc + 1) * P, :]
                ).then_inc(dmag[DGRP[dc]], 16)
            sync.dma_start(out=sel8_t[:], in_=sel8[:, :]).then_inc(dmag[4], 16)
            sync.dma_start(out=selb_t[:], in_=selb[:, :]).then_inc(dmag[4], 16)
            sync.wait_ge(exp_done, NPAIRS)
            sync.dma_start(out=out_d[:, :], in_=osb_t[:]).then_inc(dma_cnt, 16)
            sync.wait_ge(s_copied, 2)
            sync.dma_start(out=tro_d[:, :], in_=tro_t[:]).then_inc(dma_cnt, 16)

        @block.gpsimd
        def _(gp):
            # small consts first: sel1b gates the interleaved S matmuls
            gp.dma_start(out=sel1b_t[:], in_=sel1b[:, :]).then_inc(dmag[4], 16)
            gp.dma_start(out=identw_t[:], in_=identw[:, :]).then_inc(dmag[4], 16)
            for dc in range(1, NDC2, 2):
                gp.dma_start(
                    out=tw_t[dc][:], in_=Tw[dc * P : (dc + 1) * P, :]
                ).then_inc(dmag[DGRP[dc]], 16)
                gp.dma_start(
                    out=xt_t[dc][:], in_=xT[dc * P : (dc + 1) * P, :]
                ).then_inc(dmag[DGRP[dc]], 16)

        @block.tensor
        def _(tensor):
            w = _WaitTracker(tensor)

            # S k-sum for chunk c (1.0 selection), interleaved into phase 1
            # in a dedicated PSUM bank so S is ready right after the last
            # m copy instead of after a separate serial pass
            # phase 1: m^T chunks (fp8 DoubleRow: 2 contraction rows per
            # PE cell).  dc-major over the DMA arrival order with one PSUM
            # accumulator per chunk: matmuls for arrived groups run across
            # all chunks while later groups stream in, so every chunk
            # completes shortly after the final group lands
            for dc in range(NDC2):
                w.wait_ge(dmag[DGRP[dc]], DGTOT[DGRP[dc]])
                for okb in range(NCH):
                    mm = nc.tensor.matmul(
                        mb_t[okb][:, 0:W],
                        tw_t[dc][:, :, okb * P : (okb + 1) * P],
                        xt_t[dc][:, :, 0:W],
                        start=(dc == 0),
                        stop=(dc == NDC2 - 1),
                        perf_mode=mybir.MatmulPerfMode.DoubleRow,
                    )
                    if dc == NDC2 - 1:
                        mm.then_inc(mm_done, 1)
            # phase 1b: S k-sums (pss bank reused: chunk 7's copy must be
            # done, so wait for all m copies)
            w.wait_ge(dmag[4], 16)  # sel1b
            w.wait_ge(m_copied, NCH)
            for c in range(NCH):
                mm = nc.tensor.matmul(
                    pss_t[0:OF, 0:W],
                    sel1b_t[:, c * OF : (c + 1) * OF],
                    m_t[c][:, 0:W],
                    start=(c == 0),
                    stop=(c == NCH - 1),
                )
                if c == NCH - 1:
                    mm.then_inc(s_done, 1)
            # phase 2: pairwise D accumulation, halves interleaved so the
            # two PE column groups (PSUM partitions 0-63 / 64-127) overlap
            for ip in range(NPAIRS):
                dp = dp_t[ip % NDP]
                if ip >= NDP:
                    w.wait_ge(exp_done, ip - NDP + 1)
                if ip == 0:
                    w.wait_ge(s_copied, 1)
                    w.wait_ge(dmag[4], 64)  # identw/sel8/selb
                # one correction matmul covers both halves: nsful rows 0-63
                # hold -S for the first half's window, rows 64-127 for the
                # second (shifted) half's window
                nc.tensor.matmul(
                    dp[:, 0:WT],
                    identw_t[:],
                    nsful_t[:],
                    start=True,
                    stop=False,
                ).then_inc(pe_abs, 1)
                for s, (c, half) in enumerate(slots):
                    po = OF * half
                    if s in a_pos:
                        w.wait_ge(act_abs, ip * SA + a_pos[s] + 1)
                        at = abs8_t[(ip * SA + a_pos[s]) % NB8]
                        st = sel8_t
                    else:
                        w.wait_ge(dve_abs, ip * SD + d_pos[s] + 1)
                        at = absb_t[(ip * SD + d_pos[s]) % NBB]
                        st = selb_t
                    nc.tensor.matmul(
                        dp[po : po + OF, 0:WT],
                        st[:, c * OF : (c + 1) * OF],
                        at[:],
                        start=False,
                        stop=(s >= NSLOT - 2),
                    ).then_inc(pe_abs, 1)

        @block.vector
        def _(vector):
            w = _WaitTracker(vector)
            ds = 0
            nc.vector.memset(eacc_t[0][:], 0.0)
            # phase 1: copy m from PSUM; mon = -m(own cols) from the bf16
            # copy (exactness of the D diagonal requires bias == -bf16(m))
            for okb in range(NCH):
                w.wait_ge(mm_done, okb + 1)
                nc.vector.tensor_copy(m_t[okb][:, 0:W], mb_t[okb][:]).then_inc(
                    dve_self, 1
                )
                ds += 1
                w.wait_ge(dve_self, ds)
                nc.vector.tensor_scalar_mul(
                    mon_t[okb][:], m_t[okb][:, 0:ROWS], -1.0
                ).then_inc(m_copied, 1)
            # S tiles: -S in bf16 per half window (rows 0-63: cols 0..288,
            # rows 64-127: cols 32..320), with ln2 added on each half's
            # distance-8 block (last 32 cols) so the doubly-covered pairs
            # contribute exactly half from each core; exp bias columns from
            # the bf16 round-trip
            w.wait_ge(s_done, 1)
            nc.vector.tensor_scalar_mul(
                nsful_t[0:OF, 0:TRW], pss_t[0:OF, 0:TRW], -1.0
            )
            nc.vector.tensor_scalar(
                nsful_t[0:OF, TRW:WT],
                pss_t[0:OF, TRW:WT],
                -1.0,
                -LN2,
                mybir.AluOpType.mult,
                mybir.AluOpType.add,
            )
            nc.vector.tensor_scalar_mul(
                nsful_t[OF:P, 0:TRW], pss_t[0:OF, HS : HS + TRW], -1.0
            )
            nc.vector.tensor_scalar(
                nsful_t[OF:P, TRW:WT],
                pss_t[0:OF, WT:W],
                -1.0,
                -LN2,
                mybir.AluOpType.mult,
                mybir.AluOpType.add,
            )
            nc.vector.tensor_copy(stmp_t[:], pss_t[0:OF, 0:ROWS]).then_inc(
                dve_self, 1
            )
            ds += 1
            w.wait_ge(dve_self, ds)
            nc.vector.tensor_scalar_mul(
                sbias_t[0:OF, :], stmp_t[:, 0:NPAIRS], -1.0
            )
            nc.vector.tensor_scalar_mul(
                sbias_t[OF:P, :], stmp_t[:, NPAIRS:ROWS], -1.0
            ).then_inc(s_copied, 1)
            # phase 2: relu tiles via 2-scalar tensor_scalar (add, max):
            # relu(m_j - m_i) = (m_j + (-m_i)) max 0 — 2x DVE perf mode
            ALU = mybir.AluOpType

            def emit_eadd(j):
                w.wait_ge(exp_done, j + 1)
                nc.vector.tensor_tensor(
                    eacc_t[(j + 1) % 2][:],
                    eacc_t[j % 2][:],
                    esc_t[j % 2][:, HS:WT],
                    ALU.add,
                ).then_inc(eadd_done, 1)

            for ip in range(NPAIRS):
                qlast = (ip + 1) * SD - 1
                if qlast >= NBB:
                    qo = qlast - NBB
                    w.wait_ge(pe_abs, g_slot(qo // SD, dve_idx[qo % SD]) + 1)
                for n, s in enumerate(dve_idx):
                    c, half = slots[s]
                    il = half * NPAIRS + ip
                    w.wait_ge(m_copied, c + 1)
                    nc.vector.tensor_scalar(
                        absb_t[(ip * SD + n) % NBB][:],
                        m_t[c][:, HS * half : HS * half + WT],
                        mon_t[c][:, il : il + 1],
                        0.0,
                        ALU.add,
                        ALU.max,
                    ).then_inc(dve_abs, 1)
                if ip >= EDA:
                    emit_eadd(ip - EDA)
            for j in range(NPAIRS - EDA, NPAIRS):
                emit_eadd(j)
            # transpose partials out (the wait also orders the same-engine
            # read of eacc after the final E-add's write)
            w.wait_ge(eadd_done, NPAIRS)
            nc.vector.tensor_copy(tro_t[:], eacc_t[0][:]).then_inc(s_copied, 1)

        @block.scalar
        def _(scalar):
            # Software-pipelined: the exp for ip is emitted after the relu
            # tiles of ip+ED so the in-order ACT engine never stalls tile
            # production on the cross-engine exp dependency chain.
            w = _WaitTracker(scalar)
            AF = mybir.ActivationFunctionType

            def emit_exp(ip):
                w.wait_ge(s_copied, 1)
                w.wait_ge(pe_abs, (ip + 1) * MM_PER_IP)
                if ip >= 2:
                    w.wait_ge(exp_done, ip - 1)  # esc ping-pong WAW
                    w.wait_ge(eadd_done, ip - 1)  # esc consumed by E-add
                nc.scalar.activation(
                    esc_t[ip % 2][:],
                    dp_t[ip % NDP][:, 0:WT],
                    AF.Exp,
                    bias=sbias_t[:, ip : ip + 1],
                    scale=-1.0,
                    accum_out=osb_t[:, ip : ip + 1],
                ).then_inc(exp_done, 1)

            for ip in range(NPAIRS):
                nlast = (ip + 1) * SA - 1
                if nlast >= NB8:
                    no = nlast - NB8
                    w.wait_ge(pe_abs, g_slot(no // SA, act_idx[no % SA]) + 1)
                for n, s in enumerate(act_idx):
                    c, half = slots[s]
                    il = half * NPAIRS + ip
                    w.wait_ge(m_copied, c + 1)
                    nc.scalar.activation(
                        abs8_t[(ip * SA + n) % NB8][:],
                        m_t[c][:, HS * half : HS * half + WT],
                        AF.Relu,
                        bias=mon_t[c][:, il : il + 1],
                        scale=1.0,
                    ).then_inc(act_abs, 1)
                if ip >= ED:
                    emit_exp(ip - ED)
            for j in range(NPAIRS - ED, NPAIRS):
                emit_exp(j)

    return nc


def _get_nc():
    if "nc" not in _cached:
        _cached["nc"] = _build_nc()
    return _cached["nc"]


def _sel_consts():
    # sel[:, c*64:(c+1)*64][p, o] = v iff o == 8*c + p//16: chunk c's
    # partition (o', k) contributes to output row 8c + o'.  Weight 2.0
    # for the relu sums, 1.0 for the plain S k-sums; ident adds the
    # -S_j correction; fold sums the two partition halves.
    sel = np.zeros((P, NCH * OF), np.float32)
    for c in range(NCH):
        for p in range(P):
            sel[p, c * OF + 8 * c + p // KD] = 2.0
    identw = np.eye(P, dtype=np.float32)
    return (
        sel.astype(ml_dtypes.float8_e5m2),
        sel.astype(ml_dtypes.bfloat16),
        (sel * 0.5).astype(ml_dtypes.bfloat16),
        identw.astype(ml_dtypes.bfloat16),
    )


def kernel(x, T):
    global last_exec_time_ns
    x = np.ascontiguousarray(np.asarray(x, dtype=np.float32))
    T = np.ascontiguousarray(np.asarray(T, dtype=np.float32))
    assert x.shape == (B, DIM) and T.shape == (DIM, OK)

    nc = _get_nc()
    sel8_np, selb_np, sel1b_np, identw_np = _sel_consts()
    # DoubleRow interleave: dram row (dcp*128+p) = contraction rows
    # (dcp*256+2p, dcp*256+2p+1) concatenated
    T_f8 = np.ascontiguousarray(
        T.astype(ml_dtypes.float8_e5m2).reshape(NDC2 * P, 2 * OK)
    )

    in_maps = []
    for c in range(NCORES):
        idx = (c * ROWS + np.arange(W)) % B
        xT_c = np.ascontiguousarray(
            x[idx].T.astype(ml_dtypes.float8_e5m2).reshape(NDC2 * P, 2 * W)
        )
        in_maps.append(
            {
                "xT": xT_c,
                "Tw": T_f8,
                "sel8": sel8_np,
                "selb": selb_np,
                "sel1b": sel1b_np,
                "identw": identw_np,
            }
        )

    trace = os.environ.get("KERNEL_TRACE") == "1"
    if trace:
        trace = _install_ntff_hook()
        tmpdir = os.environ.get("KERNEL_TRACE_DIR") or None
        if tmpdir:
            os.makedirs(tmpdir, exist_ok=True)
    else:
        tmpdir = None
    res = run_bass_kernel_spmd(
        nc, in_maps, core_ids=list(range(NCORES)), trace=trace, tmpdir=tmpdir
    )
    last_exec_time_ns = res.exec_time_ns

    out_full = np.zeros((B, OF), np.float64)
    for c in range(NCORES):
        r = np.asarray(res.results[c]["out"], dtype=np.float64)  # [128, 32]
        blk = out_full[c * ROWS : (c + 1) * ROWS]
        blk[0:NPAIRS] += r[:OF].T
        blk[NPAIRS:ROWS] += r[OF:].T
        tr = np.asarray(res.results[c]["tro"], dtype=np.float64)  # [128, 256]
        rows0 = (c * ROWS + HS + np.arange(TRW)) % B  # first half: cols 32..288
        rows1 = (c * ROWS + 2 * HS + np.arange(TRW)) % B  # second: cols 64..320
        np.add.at(out_full, rows0, tr[:OF].T)
        np.add.at(out_full, rows1, tr[OF:].T)
    return np.concatenate([x, out_full.astype(np.float32)], axis=1)
